# revision 47
# baseline (speedup 1.0000x reference)
"""Multi-head attention (B=2, S=2048, D=1024, H=16) on 8 trn2 NeuronCores.

Sharding: core c handles batch b = c // 4 and head-group g = c % 4
(4 heads = 256 hidden columns per core).  Host sums the 4 partials per
batch and adds the linear bias terms (bo + Wo @ bv) exactly.

v4 pipeline (ACT-bound design):
  - The softmax exp on ACT is the floor (~137us: 16.8M elements at
    1/cycle/lane @1.2GHz).  Everything is scheduled to keep ACT fed.
  - Scores are row-tiled matmul PAIRS (K=64, heads at partition base
    0/64, interleaved emission) -> 2x PE concurrency (measured).
  - 8 passes of (p, hp, n) x 16 kt subcycles; per subcycle: attnV of
    kt-lag, score pair kt, ~1 filler job, one [128,1024] exp.
  - Pass order hp0 x4 then hp1 x4: xk/xq chunks are reused across p,
    so the input-DMA crunch (14MB at ~200GB/s) only binds in the first
    two passes; k(1,*)/q(1,*) projections get ~40us of slack.
  - Pass 0 defers its attnV into pass 1 (16 persistent at tiles,
    replayed one per subcycle) so pass 0 needs no xv data at all.
  - Pass-end attnV drains + norms run as jobs in the NEXT pass
    (in-order PE queue never waits on the norm chain at a boundary).
  - Projection jobs are d-halves (4 MMs, ~850ns) so no job overfills
    a subcycle.  PSUM: sc ping/pong (4 banks) + av (2) + 2 filler.
  - DMA: arrival-ordered, weights split by head-pair, descriptor-gen
    spread across the sync and scalar HWDGE rings; outT written bf16
    via per-chunk 1MB batched DMAs.
"""
import os
import sys
import types
from collections import deque

sys.path.insert(0, "/opt/trn_rl_repo")

import numpy as np

B = 2
S = 2048
D = 1024
H_PER_CORE = 4
DH = 64
JG = 256            # hidden cols per core
ND = D // 128       # 8 contraction d-tiles
NKT = S // 128      # 16 key tiles
NC_ = 4             # 512-col q chunks
QC = 512
SCALE = 1.0 / np.sqrt(DH)
RING = 5            # at ring depth (attnV lags exp by up to 4)
LAG = 4

_cache = {}


def _install_profshim():
    if "antenv.axon_hooks" in sys.modules:
        return
    try:
        from trn_agent_boot.trn_boot import _ntff_profile_via_ctypes

        hook = _ntff_profile_via_ctypes("/opt/axon/libaxon_pjrt.so")
        mod = types.ModuleType("antenv.axon_hooks")
        mod.get_axon_ntff_profile_hook = lambda: hook
        mod.set_axon_ntff_profile_hook = lambda h: None
        sys.modules["antenv.axon_hooks"] = mod
        import concourse.bass_utils as _bu

        _bu.upload_artifacts = lambda tmpdir: "local://unavailable"
    except Exception:
        pass


def build_nc():
    import concourse.bacc as bacc
    import concourse.mybir as mybir
    import concourse.tile as tile

    f32 = mybir.dt.float32
    bf16 = mybir.dt.bfloat16
    AF = mybir.ActivationFunctionType

    nc = bacc.Bacc("TRN2", target_bir_lowering=False)

    # host pre-arranges everything chunk-major/partition-major so every
    # DMA moves contiguous multi-KB lines (1KB-line column gathers were
    # descriptor-dominated: ~200 GB/s ceiling, 2-6us DGE per start)
    xqT = nc.dram_tensor("xqT", [NC_, 128, ND, QC], bf16,
                         kind="ExternalInput").ap()
    xkT = nc.dram_tensor("xkT", [NC_, 128, ND, QC], bf16,
                         kind="ExternalInput").ap()
    xvT = nc.dram_tensor("xvT", [NC_, 128, ND, QC], bf16,
                         kind="ExternalInput").ap()
    wqT = nc.dram_tensor("wqT", [128, ND, JG], bf16,
                         kind="ExternalInput").ap()
    wkT = nc.dram_tensor("wkT", [128, ND, JG], bf16,
                         kind="ExternalInput").ap()
    wvT = nc.dram_tensor("wvT", [128, ND, JG], bf16,
                         kind="ExternalInput").ap()
    woT = nc.dram_tensor("woT", [128, 2, ND, 128], bf16,
                         kind="ExternalInput").ap()
    bq = nc.dram_tensor("bq", [128, 2], f32, kind="ExternalInput").ap()
    bk = nc.dram_tensor("bk", [128, 2], f32, kind="ExternalInput").ap()
    outT = nc.dram_tensor("outT", [NC_, 128, ND, QC], bf16,
                          kind="ExternalOutput").ap()

    with tile.TileContext(nc) as tc:
        with (
            tc.tile_pool(name="xs", bufs=1) as x_pool,
            tc.tile_pool(name="wts", bufs=1) as w_pool,
            tc.tile_pool(name="qkv", bufs=1) as qkv_pool,
            tc.tile_pool(name="attn", bufs=1) as attn_pool,
            tc.tile_pool(name="small", bufs=1) as small_pool,
            tc.tile_pool(name="nrm", bufs=1) as nrm_pool,
            tc.tile_pool(name="oev", bufs=1) as oev_pool,
            tc.tile_pool(name="psum", bufs=1, space="PSUM") as pp,
        ):
            # ---- tiles for DMA targets -----------------------------------
            wk_t = w_pool.tile([128, ND, JG], bf16, tag="wk")
            wq_t = w_pool.tile([128, ND, JG], bf16, tag="wq")
            wv_t = w_pool.tile([128, ND, JG], bf16, tag="wv")
            wo_t = w_pool.tile([128, 2, ND, 128], bf16, tag="wo")
            bq_t = small_pool.tile([128, 2], f32, tag="bq")
            bk_t = small_pool.tile([128, 2], f32, tag="bk")

            def w_half(dst, dram, hp, eng):
                eng.dma_start(
                    dst[:, :, hp * 128:(hp + 1) * 128],
                    dram[:, :, hp * 128:(hp + 1) * 128],
                )

            def x_chunk(name, dram, c, eng):
                t = x_pool.tile([128, ND, QC], bf16, tag=f"{name}{c}",
                                name=f"{name}{c}")
                eng.dma_start(t[:], dram[c])
                return t

            def x_chunk_half(name, dram, c, h, eng):
                t = x_pool.tile([128, ND // 2, QC], bf16,
                                tag=f"{name}{c}h{h}", name=f"{name}{c}h{h}")
                eng.dma_start(t[:], dram[c, :, h * 4:(h + 1) * 4, :])
                return t

            # ---- DMA: arrival-ordered, DGE spread over sync+scalar -------
            xk_c = [None] * NC_
            xq_c = [None] * NC_
            xv_c = [None] * NC_
            w_half(wk_t, wkT, 0, nc.scalar)
            xk0h = [x_chunk_half("xk", xkT, 0, 0, nc.sync),
                    x_chunk_half("xk", xkT, 0, 1, nc.scalar)]
            w_half(wq_t, wqT, 0, nc.sync)
            xq0h = [x_chunk_half("xq", xqT, 0, 0, nc.scalar),
                    x_chunk_half("xq", xqT, 0, 1, nc.sync)]
            nc.scalar.dma_start(bq_t[:], bq)
            nc.sync.dma_start(bk_t[:], bk)
            # everything past this point rides the sync ring: the scalar
            # HWDGE ring shares the ACT sequencer FIFO, so queuing bulk
            # DMAs there delays every subsequent exp dispatch
            xk_c[1] = x_chunk("xk", xkT, 1, nc.sync)
            w_half(wv_t, wvT, 0, nc.sync)
            w_half(wv_t, wvT, 1, nc.sync)
            xv_c[0] = x_chunk("xv", xvT, 0, nc.sync)
            xk_c[2] = x_chunk("xk", xkT, 2, nc.sync)
            xk_c[3] = x_chunk("xk", xkT, 3, nc.sync)
            xv_c[1] = x_chunk("xv", xvT, 1, nc.sync)
            xq_c[1] = x_chunk("xq", xqT, 1, nc.sync)
            xv_c[2] = x_chunk("xv", xvT, 2, nc.sync)
            xv_c[3] = x_chunk("xv", xvT, 3, nc.sync)
            xq_c[2] = x_chunk("xq", xqT, 2, nc.sync)
            w_half(wk_t, wkT, 1, nc.sync)
            w_half(wq_t, wqT, 1, nc.sync)
            xq_c[3] = x_chunk("xq", xqT, 3, nc.sync)
            nc.sync.dma_start(wo_t[:], woT)

            def xk_d(c, d):
                if c == 0:
                    return xk0h[d // 4][:, d % 4, :]
                return xk_c[c][:, d, :]

            def xq_d(c, d):
                if c == 0:
                    return xq0h[d // 4][:, d % 4, :]
                return xq_c[c][:, d, :]

            # ---- persistent SBUF -----------------------------------------
            q_t = [qkv_pool.tile([128, S], bf16, tag=f"qt{m}", name=f"qt{m}")
                   for m in range(2)]
            k_t = [qkv_pool.tile([128, S], bf16, tag=f"kt{m}", name=f"kt{m}")
                   for m in range(2)]
            v_t = [qkv_pool.tile([128, H_PER_CORE, DH + 1], bf16,
                                 tag=f"v{s}", name=f"v{s}")
                   for s in range(NKT)]
            ao_t = [qkv_pool.tile([128, S], bf16, tag=f"ao{m}", name=f"ao{m}")
                    for m in range(2)]
            ones4 = small_pool.tile([128, H_PER_CORE], f32, tag="ones4")
            nc.vector.memset(ones4[:], 1.0)

            at_ring = [attn_pool.tile([128, 2 * QC], bf16, tag=f"at{j}",
                                      name=f"at{j}") for j in range(RING)]
            at_A = [attn_pool.tile([128, 2 * QC], bf16, tag=f"atA{j}",
                                   name=f"atA{j}") for j in range(NKT)]

            # ---- projection / wo jobs ------------------------------------
            pj_state = {"i": 0, "cur": {}}

            def pj_tile(shape, nm, pin=None):
                if pin is None:
                    pin = pj_state["i"] = pj_state["i"] ^ 1
                return pp.tile(shape, f32, tag=f"pj{pin}", name=nm)

            def _proj_half(key, w_full, xd_fn, hp, c, h, dst, bias, pin=None):
                """half h of an 8-deep accumulation; evac on h==1."""
                if h == 0:
                    pj_state["cur"][key] = pj_tile([128, QC], key, pin=pin)
                ps = pj_state["cur"][key]
                for d in range(4 * h, 4 * h + 4):
                    nc.tensor.matmul(
                        ps[:], w_full[:, d, hp * 128:(hp + 1) * 128],
                        xd_fn(c, d),
                        start=(d == 0), stop=(d == ND - 1),
                    )
                if h == 1:
                    nc.vector.tensor_scalar_add(
                        dst[hp][:, c * QC:(c + 1) * QC], ps[:],
                        bias[:, hp:hp + 1],
                    )

            def q_half(hp, c, h, pin=None):
                _proj_half(f"qp{hp}{c}", wq_t, xq_d, hp, c, h, q_t, bq_t,
                           pin=pin)

            def k_half(hp, c, h):
                _proj_half(f"kp{hp}{c}", wk_t, xk_d, hp, c, h, k_t, bk_t)

            def v_tile(s):
                ps = pj_tile([128, JG], f"vp{s}")
                c, r = divmod(s, 4)
                for d in range(ND):
                    nc.tensor.matmul(
                        ps[:], xv_c[c][:, d, r * 128:(r + 1) * 128],
                        wv_t[:, d, :],
                        start=(d == 0), stop=(d == ND - 1),
                    )
                nc.vector.tensor_copy(
                    v_t[s][:, :, 0:DH],
                    ps[:].rearrange("p (h d) -> p h d", d=DH),
                )
                nc.vector.tensor_copy(v_t[s][:, :, DH], ones4[:])

            ot_big = {}

            def wo_im(p, c, im):
                ps = pj_tile([128, QC], f"wo{p}{c}{im}")
                cols = slice(p * 1024 + (c % 2) * QC,
                             p * 1024 + (c % 2) * QC + QC)
                for jk in range(2):
                    nc.tensor.matmul(
                        ps[:], wo_t[:, jk, im, :], ao_t[jk][:, cols],
                        start=(jk == 0), stop=(jk == 1),
                    )
                key = (p, c)
                if key not in ot_big:
                    ot_big[key] = oev_pool.tile(
                        [128, ND, QC], bf16, tag="ot0", name=f"ot{p}{c}")
                nc.vector.tensor_copy(ot_big[key][:, im, :], ps[:])

            def wo_flush(p, c, part=None):
                if part is None:
                    nc.sync.dma_start(outT[c], ot_big[(p, c)][:])
                else:
                    nc.sync.dma_start(
                        outT[c, :, part * 2:(part + 1) * 2, :],
                        ot_big[(p, c)][:, part * 2:(part + 1) * 2, :],
                    )

            # ---- attention machinery -------------------------------------
            sc_tiles = [pp.tile([128, 2 * QC], f32, tag=f"sc{j}",
                                name=f"sc{j}") for j in range(2)]
            av_t = {hh: pp.tile([DH + 1, QC], f32, tag=f"av{hh}",
                                name=f"av{hh}") for hh in range(2)}

            def sc_pair(hp, kt, cols, sl):
                sc = sc_tiles[sl]
                for hh in range(2):
                    nc.tensor.matmul(
                        sc[:, hh * QC:(hh + 1) * QC],
                        k_t[hp][hh * DH:(hh + 1) * DH,
                                kt * 128:(kt + 1) * 128],
                        q_t[hp][hh * DH:(hh + 1) * DH, cols],
                        start=True, stop=True,
                    )

            def av_pair(hp, kt, at, avd, first, last):
                for hh in range(2):
                    nc.tensor.matmul(
                        avd[hh][:],
                        v_t[kt][:, hp * 2 + hh, :],
                        at[:, hh * QC:(hh + 1) * QC],
                        start=first, stop=last,
                    )

            def norm1(hp, hh, cols, av_ap, nm):
                dn = nrm_pool.tile([1, QC], f32, tag=f"dn{hh}",
                                   name=f"dn{nm}{hh}")
                nc.vector.tensor_copy(dn[:], av_ap[DH:DH + 1, :])
                rc = nrm_pool.tile([1, QC], f32, tag=f"rc{hh}",
                                   name=f"rc{nm}{hh}")
                nc.vector.reciprocal_approx_fast(rc[:], dn[:])
                rb = nrm_pool.tile([DH, QC], f32, tag=f"rb{hh}",
                                   name=f"rb{nm}{hh}")
                nc.gpsimd.partition_broadcast(rb[:], rc[:])
                nc.vector.tensor_mul(
                    ao_t[hp][hh * DH:(hh + 1) * DH, cols],
                    av_ap[0:DH, :], rb[:],
                )

            def norms(hp, cols, avd, nm):
                for hh in range(2):
                    norm1(hp, hh, cols, avd(hh), nm)

            # passes: p-major so pass 1 reuses all of pass 0's projections
            passes = [(0, 0, 0), (0, 0, 1), (1, 0, 0), (1, 0, 1),
                      (0, 1, 0), (0, 1, 1), (1, 1, 0), (1, 1, 1)]

            def pass_cols(pi):
                p, _hp, n = passes[pi]
                a = p * 1024 + n * QC
                return slice(a, a + QC)

            def ring_at(pi, j):
                return at_ring[(pi * NKT + j) % RING]

            # drain jobs for pass pi (attnV tail + norms), run in pass pi+1
            def drain_av(pi, j):
                _p, hp, _n = passes[pi]
                av_pair(hp, j, ring_at(pi, j), av_t,
                        first=(j == 0), last=(j == NKT - 1))

            def drain_norm(pi):
                _p, hp, _n = passes[pi]
                norms(hp, pass_cols(pi), lambda hh: av_t[hh][:], f"p{pi}")

            # pass-0's attnV is deferred entirely (its at tiles persist in
            # at_A); replayed one HEAD at a time through a single pj0-tag
            # accumulator bank (one open PSUM group per bank; pj1 stays
            # free for projections)
            avR = {}

            def replay_round(hh, jlo):
                """head hh of pass 0, full 512 cols; one job = 2 kt."""
                if jlo == 0:
                    avR[hh] = pj_tile([DH + 1, QC], f"avR{hh}", pin=0)
                for j in (jlo, jlo + 1):
                    nc.tensor.matmul(
                        avR[hh][:],
                        v_t[j][:, hh, :],
                        at_A[j][:, hh * QC:(hh + 1) * QC],
                        start=(j == 0), stop=(j == NKT - 1),
                    )

            def replay_norm(hh):
                norm1(0, hh, pass_cols(0), avR[hh][:], f"pA{hh}")

            # ---- filler plans --------------------------------------------
            def D_(pi):
                return ([lambda j=j: drain_av(pi, j)
                         for j in range(NKT - LAG, NKT)]
                        + [lambda: drain_norm(pi)])

            plan = {
                # pass 0 defers attnV -> v tiles not needed until pass 1
                0: [lambda: k_half(0, 1, 0), lambda: k_half(0, 1, 1),
                    lambda: k_half(0, 2, 0), lambda: k_half(0, 2, 1),
                    lambda: k_half(0, 3, 0), lambda: k_half(0, 3, 1),
                    lambda: q_half(0, 1, 0), lambda: q_half(0, 1, 1)]
                   + [lambda s=s: v_tile(s) for s in range(0, 8)],
                # pass 1: remaining v tiles, then replay round A (claims
                # only the pj0 bank; round B + norms ride pass 2)
                1: [lambda s=s: v_tile(s) for s in range(8, NKT)]
                   + [lambda j=j: replay_round(0, j)
                      for j in range(0, NKT, 2)],
                2: D_(1)
                   + [lambda: q_half(0, 3, 0, pin=1),
                      lambda: q_half(0, 3, 1, pin=1),
                      lambda: replay_norm(0)]
                   + [lambda j=j: replay_round(1, j)
                      for j in range(0, NKT, 2)]
                   + [lambda: replay_norm(1)],
                3: D_(2) + [lambda: k_half(1, 0, 0), lambda: k_half(1, 0, 1),
                            lambda: k_half(1, 1, 0), lambda: k_half(1, 1, 1),
                            lambda: k_half(1, 2, 0), lambda: k_half(1, 2, 1),
                            lambda: q_half(1, 0, 0), lambda: q_half(1, 0, 1)],
                # k(1,3) feeds kt 12-15 of pass 4 itself -> JIT-legal there
                4: D_(3) + [lambda: k_half(1, 3, 0), lambda: k_half(1, 3, 1),
                            lambda: q_half(1, 1, 0), lambda: q_half(1, 1, 1)],
                5: D_(4) + [lambda: q_half(1, 2, 0), lambda: q_half(1, 2, 1)]
                   + [lambda im=im: wo_im(0, 0, im) for im in range(ND)]
                   + [lambda: wo_flush(0, 0)],
                6: D_(5) + [lambda: q_half(1, 3, 0), lambda: q_half(1, 3, 1)]
                   + [lambda im=im: wo_im(0, 1, im) for im in range(ND)]
                   + [lambda: wo_flush(0, 1)],
                7: D_(6) + [lambda im=im: wo_im(1, 2, im) for im in range(ND)]
                   + [lambda: wo_flush(1, 2)],
            }
            spill = {}
            # pass-2 preamble: q(0,2) feeds pass 2's own scores; pinned to
            # the pj1 bank so it does not touch the live replay accumulator
            preamble = {2: [lambda: q_half(0, 2, 0, pin=1),
                            lambda: q_half(0, 2, 1, pin=1)]}

            # ---- pre-phase -----------------------------------------------
            # dummy exp pulls the ~2.7us ACT table load off the critical path
            dummy = small_pool.tile([128, 4], f32, tag="dmy")
            dummy2 = small_pool.tile([128, 4], f32, tag="dmy2")
            nc.vector.memset(dummy[:], 1.0)
            nc.scalar.activation(dummy2[:], dummy[:], AF.Exp)

            # PE warmup: ~8us of dummy matmuls during the initial DMA wait
            # ramps the p-state to 2.4GHz before the first real projections
            warm = small_pool.tile([128, QC], bf16, tag="warm")
            nc.vector.memset(warm[:], 0.0)

            def warmup(n):
                for i in range(n):
                    nc.tensor.matmul(
                        sc_tiles[0][:, 0:QC], warm[:, 0:128], warm[:],
                        start=True, stop=True,
                    )

            warmup(12)
            k_half(0, 0, 0)
            k_half(0, 0, 1)
            q_half(0, 0, 0)
            q_half(0, 0, 1)

            # ---- the pipeline --------------------------------------------
            for pi, (p, hp, n) in enumerate(passes):
                cols = pass_cols(pi)
                jobs = deque(plan[pi])
                for fn in preamble.get(pi, []):
                    fn()
                defer = (pi == 0)

                for kt in range(NKT):
                    # order [sc][job][av][exp]: a drain_norm job (reads
                    # av_t) must precede this pass's own av(0) (writes)
                    sc_pair(hp, kt, cols, kt % 2)
                    if jobs:
                        jobs.popleft()()
                    if not defer and kt >= LAG:
                        j = kt - LAG
                        av_pair(hp, j, ring_at(pi, j), av_t,
                                first=(j == 0), last=False)
                    dst = at_A[kt] if defer else ring_at(pi, kt)
                    nc.scalar.activation(
                        dst[:], sc_tiles[kt % 2][:], AF.Exp,
                        scale=float(SCALE),
                    )

                while jobs:
                    jobs.popleft()()
                for fn in spill.get(pi, []):
                    fn()

            # ---- tail ----------------------------------------------------
            for j in range(NKT - LAG, NKT):
                drain_av(7, j)
            drain_norm(7)
            # keep the PE hot through the norm-chain latency
            warmup(8)
            for im in range(ND):
                wo_im(1, 3, im)
                if im % 2 == 1:
                    wo_flush(1, 3, part=im // 2)

    nc.compile()
    return nc


def _get_nc():
    if "nc" not in _cache:
        _cache["nc"] = build_nc()
    return _cache["nc"]


def kernel(q, k, v, Wq, bq, Wk, bk, Wv, bv, Wo, bo, **_unused):
    import ml_dtypes
    from concourse.bass_utils import run_bass_kernel_spmd

    bf = ml_dtypes.bfloat16
    q = np.asarray(q, dtype=np.float32)
    k = np.asarray(k, dtype=np.float32)
    v = np.asarray(v, dtype=np.float32)
    Wq = np.asarray(Wq, dtype=np.float32)
    Wk = np.asarray(Wk, dtype=np.float32)
    Wv = np.asarray(Wv, dtype=np.float32)
    Wo = np.asarray(Wo, dtype=np.float32)
    bq = np.asarray(bq, dtype=np.float32)
    bk = np.asarray(bk, dtype=np.float32)
    bv = np.asarray(bv, dtype=np.float32)
    bo = np.asarray(bo, dtype=np.float32)

    nc = _get_nc()

    def chunkx(xb):
        # [S, D] -> x.T chunk-major [NC, 128, ND, QC]:
        # arr[c, p, d, q] = x.T[d*128+p, c*QC+q]
        xT_ = xb.T.reshape(ND, 128, NC_, QC).transpose(2, 1, 0, 3)
        return np.ascontiguousarray(xT_).astype(bf)

    def packw(wT):
        # [D, JG] (= W.T slice) -> [128, ND, JG]
        return np.ascontiguousarray(
            wT.reshape(ND, 128, JG).transpose(1, 0, 2)).astype(bf)

    xT = {b: {} for b in range(B)}
    for b in range(B):
        xT[b]["q"] = chunkx(q[b])
        xT[b]["k"] = chunkx(k[b])
        xT[b]["v"] = chunkx(v[b])

    wslices = []
    for g in range(4):
        J = slice(g * JG, (g + 1) * JG)
        woTg = Wo[:, J].T  # [JG, D]
        wslices.append({
            "wqT": packw(Wq.T[:, J]),
            "wkT": packw(Wk.T[:, J]),
            "wvT": packw(Wv.T[:, J]),
            # [JG, D] -> [128, 2, ND, 128]
            "woT": np.ascontiguousarray(
                woTg.reshape(2, 128, ND, 128).transpose(1, 0, 2, 3)
            ).astype(bf),
            "bq": np.ascontiguousarray(
                bq[J].reshape(2, 128).T),
            "bk": np.ascontiguousarray(
                bk[J].reshape(2, 128).T),
        })

    in_maps = []
    for c in range(8):
        b, g = c // 4, c % 4
        m = {"xqT": xT[b]["q"], "xkT": xT[b]["k"], "xvT": xT[b]["v"]}
        m.update(wslices[g])
        in_maps.append(m)

    trace = bool(int(os.environ.get("KERNEL_TRACE", "0")))
    if trace:
        _install_profshim()
    res = run_bass_kernel_spmd(
        nc, in_maps, core_ids=list(range(8)), trace=trace
    )
    _cache["exec_time_ns"] = res.exec_time_ns
    parts = [np.asarray(r["outT"], dtype=np.float32) for r in res.results]

    const_row = bo + Wo @ bv
    out = np.empty((B, S, D), dtype=np.float32)
    for b in range(B):
        acc = parts[4 * b]
        for g in range(1, 4):
            acc = acc + parts[4 * b + g]
        # [NC, 128, ND, QC] -> [D, S] -> out rows [S, D]
        acc = acc.transpose(2, 1, 0, 3).reshape(D, S)
        out[b] = acc.T + const_row
    return out
